# revision 64
# baseline (speedup 1.0000x reference)
"""GCN message-passing kernel for Trainium2 (8 NeuronCores, batch-parallel).

Model (see problem reference): two GCN layers over a fixed random graph
(N=1024 nodes, E=8192 directed edges, topology shared by all B=256
samples), LeakyReLU activations, global mean pool, Linear(64,128)+LeakyReLU.

Strategy
--------
Shared topology => GCN aggregation is a fixed linear operator per sample:
agg = A_hat @ h with A_hat = D^-1/2 (A+I) D^-1/2.  For layer 1, A_hat is
split into the *binary* matrix A+I (small integer counts - exactly
representable in fp8) and diagonal dinv scalings folded into neighbouring
ops (positive homogeneity of LeakyReLU lets dinv commute through the
activation); for layer 2 the dst-side dinv is folded into a second fp8
adjacency (~3% entry error that the mean pool averages away).  All
aggregation contractions and the Z1 weight matmul (via an fp8 hi+lo split
of BD1) run as fp8 DoubleRow matmuls on the PE: 2 k-tiles per pass at 0.5
cycles/col, ~4x the fp32r rate.

Data-parallel: 8 cores x 32 samples.  Per core, activations live in SBUF
as N-layout [node partition, (b, f) free] or T-layout [(b, f) partition,
node free]; feature matmuls use host-built block-diagonal weights so the
layouts flip for free between stages (zero explicit transposes):

  AGG0T[96,1024]   = sum_kp DR(H0'[kp], A01[kp])      (x' = dinv*x, fp8)
  AGG0T           *= dinv[dst]^2 -> fp8 x2             (DVE, PSUM->SBUF)
  Z1   [1024,2048] = DR(AGG0T_slices, BD1 hi+lo)       (fp8 DoubleRow)
  act1             = LeakyReLU(Z1) -> fp8              (ACT)
  AGG2T[2048,1024] = sum_kp DR(act1[kp], A01*dinv[kp]) (fp8 DoubleRow)
  a2               = PSUM -> SBUF copy                 (DVE)
  Z2T              = BD2.T @ a2                        (fp32r)
  G(pool)          = sum LeakyReLU(Z2T)                (ACT / DVE max+min)
  Y                = LeakyReLU(G_slices.T @ (Wp/1024)) (fp32r)

The layer-1-weight and layer-2 stages are fused into one software-
pipelined PE stream over four 512-column groups, with the elementwise
work balanced across ACT and DVE (GPSIMD cannot read PSUM on TRN2, and
the s2s2d2 ops cannot read PSUM twice, which pins LeakyReLU-from-PSUM to
ACT or a DVE max/min accumulation pair).
"""

import numpy as np

B = 256
N = 1024
F_IN = 3
D1 = 64
D2 = 64
D_OUT = 128
NEG = 0.01
NCORES = 8
BS = B // NCORES          # 32 samples per core
NT = N // 128             # 8 node tiles
BD1_COLS = BS * D1        # 2048
NCHUNK = BD1_COLS // 128  # 16 (b,d)-chunks of 128

# h2 engine rotation: the GPSIMD/Pool engine cannot read PSUM and the
# s2s2d2 STT op cannot read PSUM twice, so LeakyReLU straight out of
# PSUM runs either on ACT (Prelu, one pass) or on DVE as a max/min
# accumulation pair (g = sum max(z,0) + NEG * sum min(z,0)).
H2_PATTERN = ["act", "act", "dve"]


def _build_host_constants(edge_index, W1, b1, W2, b2, Wp, bp):
    """Binary adjacency (fp8-exact), dinv scalings, block-diag weights."""
    from concourse import mybir
    f8np = mybir.dt.np(mybir.dt.float8e4)

    src = np.asarray(edge_index[0], dtype=np.int64)
    dst = np.asarray(edge_index[1], dtype=np.int64)
    deg = np.bincount(dst, minlength=N).astype(np.float32) + 1.0
    dinv = (1.0 / np.sqrt(deg)).astype(np.float32)

    # a01[s, d] = #edges(s->d) + [s == d]; small ints, exact in fp8.
    a01 = np.zeros((N, N), dtype=np.float32)
    np.add.at(a01, (src, dst), 1.0)
    a01[np.arange(N), np.arange(N)] += 1.0
    a01_8 = np.ascontiguousarray(a01.astype(f8np))

    W1 = np.asarray(W1, dtype=np.float32)
    W2 = np.asarray(W2, dtype=np.float32)
    Wp = np.asarray(Wp, dtype=np.float32)

    # BD1[(b, f), (b, d)] = W1[f, d] on the block diagonal. [96, 2048]
    bd1 = np.zeros((BS * F_IN, BS * D1), dtype=np.float32)
    for b in range(BS):
        bd1[b * F_IN:(b + 1) * F_IN, b * D1:(b + 1) * D1] = W1
    # fp8 hi/lo split of BD1: hi + lo reproduces BD1 to ~0.4% so the Z1
    # matmul can run as one fp8 DoubleRow pass (contract hi and lo
    # k-copies in a single instruction at 0.5 cycles/col).
    bd1_hi = bd1.astype(f8np).astype(np.float32)
    bd1hl = np.stack([bd1_hi, bd1 - bd1_hi], axis=1)  # [96, 2, 2048]
    bd1hl_8 = np.ascontiguousarray(bd1hl.astype(f8np))
    # BD2 = blockdiag(W2, W2): one 128-row chunk covers 2 samples. [128, 128]
    bd2 = np.zeros((2 * D1, 2 * D2), dtype=np.float32)
    bd2[:D1, :D2] = W2
    bd2[D1:, D2:] = W2
    # Mean pool folded into the projection weight; stacked twice so both
    # halves of the pooled G tile have a matching rhs at the same base
    # partition.
    wp_s = np.vstack([Wp / float(N), Wp / float(N)]).astype(np.float32)

    consts = {
        "a01": a01_8,
        # layer-2 adjacency with the dst-side dinv folded in (fp8, ~3%
        # entry error; the mean pool averages it away)
        "a2d8": np.ascontiguousarray((a01 * dinv[None, :]).astype(f8np)),
        "dinv": dinv,
        "dinv2_bc": np.ascontiguousarray(
            np.broadcast_to((dinv * dinv)[None, :], (128, N))),
        "bd1hl": bd1hl_8,
        "bd2": bd2,
        "wp_s": wp_s,
        # bias rows (all zero for this problem; kept for generality)
        "b1row": np.tile(np.asarray(b1, np.float32), BS)[None, :],   # [1, 2048]
        "b2col": np.tile(np.asarray(b2, np.float32), 2)[None, :],    # [1, 128]
        "bprow": np.asarray(bp, np.float32)[None, :],                # [1, 128]
    }
    return consts


def _prep_x8(x_core, dinv):
    """x[b, 3n+f] -> fp8 tile [128, (nt b f)] pre-scaled by dinv[node]."""
    from concourse import mybir
    f8np = mybir.dt.np(mybir.dt.float8e4)
    xr = x_core.reshape(BS, N, F_IN) * dinv[None, :, None]
    x8 = xr.reshape(BS, NT, 128, F_IN).transpose(2, 1, 0, 3)
    return np.ascontiguousarray(x8.reshape(128, NT * BS * F_IN).astype(f8np))


_PROGRAM_CACHE = {}


def _build_program(with_b1, with_b2, with_bp, reps=1):
    key = (with_b1, with_b2, with_bp, reps)
    if key in _PROGRAM_CACHE:
        return _PROGRAM_CACHE[key]

    import concourse.mybir as mybir
    import concourse.tile as tile
    from concourse import bacc

    f32 = mybir.dt.float32
    f32r = mybir.dt.float32r
    f8 = mybir.dt.float8e4

    # Bacc (not raw Bass): its compile() runs move_matmul_waits_to_ldweights
    # + generate_event_semaphores, which split sync waits down to the 1-per-
    # instruction hardware limit (self-loading fp32r matmuls hit this).
    nc = bacc.Bacc(trn_type="TRN2", target_bir_lowering=False, debug=False)

    x8_t = nc.dram_tensor("x8", [128, NT * BS * F_IN], f8,
                          kind="ExternalInput").ap()
    at_t = nc.dram_tensor("a01", [N, N], f8, kind="ExternalInput").ap()
    dbc_t = nc.dram_tensor("a2d8", [N, N], f8, kind="ExternalInput").ap()
    dcol_t = nc.dram_tensor("dinv2_bc", [128, N], f32,
                            kind="ExternalInput").ap()
    bd1_t = nc.dram_tensor("bd1hl", [BS * F_IN, 2, BS * D1], f8,
                           kind="ExternalInput").ap()
    bd2_t = nc.dram_tensor("bd2", [128, 128], f32r,
                           kind="ExternalInput").ap()
    wp_t = nc.dram_tensor("wp_s", [2 * D2, D_OUT], f32r,
                          kind="ExternalInput").ap()
    b1_t = nc.dram_tensor("b1row", [1, BS * D1], f32r,
                          kind="ExternalInput").ap() if with_b1 else None
    b2_t = nc.dram_tensor("b2col", [1, 128], f32r,
                          kind="ExternalInput").ap() if with_b2 else None
    bp_t = nc.dram_tensor("bprow", [1, D_OUT], f32r,
                          kind="ExternalInput").ap() if with_bp else None
    y_t = nc.dram_tensor("y", [BS, D_OUT], f32, kind="ExternalOutput").ap()

    tensors = (x8_t, at_t, dbc_t, dcol_t, bd1_t, bd2_t, wp_t,
               b1_t, b2_t, bp_t, y_t)

    with tile.TileContext(nc) as tc:
        if reps > 1:
            with tc.For_i(0, reps, 1):
                _emit_body(nc, tc, mybir, tensors, with_b1, with_b2, with_bp)
        else:
            _emit_body(nc, tc, mybir, tensors, with_b1, with_b2, with_bp)

    nc.compile()
    _PROGRAM_CACHE[key] = nc
    return nc


def _emit_body(nc, tc, mybir, tensors, with_b1, with_b2, with_bp):
    from contextlib import ExitStack

    (x8_t, at_t, dbc_t, dcol_t, bd1_t, bd2_t, wp_t,
     b1_t, b2_t, bp_t, y_t) = tensors

    f32 = mybir.dt.float32
    f32r = mybir.dt.float32r
    f8 = mybir.dt.float8e4
    DR = mybir.MatmulPerfMode.DoubleRow
    ALU = mybir.AluOpType
    PRELU = mybir.ActivationFunctionType.Prelu
    ENG = {"act": nc.scalar, "dve": nc.vector, "pool": nc.gpsimd}

    with ExitStack() as es:
        const = es.enter_context(tc.tile_pool(name="const", bufs=1))
        work = es.enter_context(tc.tile_pool(name="work", bufs=1))

        at_sb = const.tile([128, NT, N], f8)        # A01 [src-part, kt, dst]
        at2_sb = const.tile([128, NT, N], f8)       # A01 * dinv[dst], fp8
        dinv2_sb = const.tile([128, N], f32)        # dinv^2 bcast over parts
        bd1_sb = const.tile([BS * F_IN, 2, BS * D1], f8)   # hi/lo pair
        bd2_sb = const.tile([128, 128], f32r)
        wp_sb = const.tile([2 * D2, D_OUT], f32r)
        h0_sb = work.tile([128, NT, BS, F_IN], f8)  # dinv*x as [node, (b,f)]
        agg0t_sb = work.tile([BS * F_IN, 2, N], f8)  # two k-copies for DR
        act1_sb = work.tile([128, NT, BD1_COLS], f8)
        gp_sb = work.tile([128, 2, NCHUNK], f32)   # sum LReLU | sum max(z,0)
        gn_sb = work.tile([128, 2, NCHUNK], f32)   # sum min(z,0) (DVE path)
        gf_sb = work.tile([128, 2, NCHUNK], f32)
        gr_sb = work.tile([128, NCHUNK], f32r)
        ye_sb = work.tile([BS // 2, D_OUT], f32)
        yo_sb = work.tile([BS // 2, D_OUT], f32)
        if with_b1:
            b1_sb = const.tile([1, BS * D1], f32r)
        if with_b2:
            b2_sb = const.tile([1, 128], f32r)
        if with_bp:
            bp_sb = const.tile([1, D_OUT], f32r)
        if with_b1 or with_b2 or with_bp:
            ones_f = const.tile([1, 512], f32)
            ones_sb = const.tile([1, 512], f32r)
            nc.any.memset(ones_f[:], 1.0)
            nc.vector.tensor_copy(ones_sb[:], ones_f[:])

        # ---- loads: everything DMAs straight into its operand tile.
        # SP feeds the PE-critical stream in consumption order (x8, A01
        # tiles, then bd1 group slices); ACT's queue brings the dinv rows
        # and small weights before its first elementwise piece is due.
        # Pool/DVE queues stay free for elementwise work. ----
        x8_r = x8_t.rearrange("p (kt b f) -> p kt b f", kt=NT, b=BS, f=F_IN)
        nc.sync.dma_start(out=h0_sb[:], in_=x8_r)
        at_r = at_t.rearrange("(kt p) d -> p kt d", kt=NT, p=128)
        for k in range(NT):
            nc.sync.dma_start(out=at_sb[:, k, :], in_=at_r[:, k, :])
        for grp in range(4):
            cs = slice(grp * 512, (grp + 1) * 512)
            nc.sync.dma_start(out=bd1_sb[:, :, cs], in_=bd1_t[:, :, cs])
        at2_r = dbc_t.rearrange("(kt p) d -> p kt d", kt=NT, p=128)
        for k in range(NT):
            nc.gpsimd.dma_start(out=at2_sb[:, k, :], in_=at2_r[:, k, :])
        nc.scalar.dma_start(out=dinv2_sb[:], in_=dcol_t)
        nc.scalar.dma_start(out=bd2_sb[:], in_=bd2_t)
        nc.scalar.dma_start(out=wp_sb[:], in_=wp_t)
        if with_b1:
            nc.scalar.dma_start(out=b1_sb[:], in_=b1_t)
        if with_b2:
            nc.scalar.dma_start(out=b2_sb[:], in_=b2_t)
        if with_bp:
            nc.scalar.dma_start(out=bp_sb[:], in_=bp_t)
        # g2's DVE min-accumulation slots must start from zero for the
        # chunks whose LeakyReLU runs fully on ACT (they never write gN).
        nc.vector.memset(gn_sb[:], 0.0)

        # ---- layer 1 aggregation: AGG0T = sum_kp H0'[kp].T @ A01[kp],
        #      then *= dinv[dst] on the PSUM->SBUF copy ----
        with tc.tile_pool(name="ps_agg0", bufs=1, space="PSUM") as ps_agg0:
            agg0t_ps = ps_agg0.tile([BS * F_IN, N], f32)
            for kp in range(NT // 2):
                for n in range(2):
                    nc.tensor.matmul(
                        agg0t_ps[:, n * 512:(n + 1) * 512],
                        h0_sb[:, 2 * kp:2 * kp + 2, :, :],
                        at_sb[:, 2 * kp:2 * kp + 2, n * 512:(n + 1) * 512],
                        start=(kp == 0), stop=(kp == NT // 2 - 1),
                        perf_mode=DR,
                    )
            # dinv^2: one dinv is layer 1's dst scaling, the other
            # premultiplies layer 2's source side (pulled through the
            # LeakyReLU by positive homogeneity), so act1 below needs no
            # per-partition scale and can run on any engine.  Written
            # twice (fp8) so the Z1 DoubleRow matmul sees the two
            # k-copies its hi/lo weight split contracts against; split in
            # halves so the first z1 matmul unblocks sooner.
            for j in range(2):
                for half in range(2):
                    hs = slice(half * 512, (half + 1) * 512)
                    nc.vector.tensor_tensor(agg0t_sb[:, j, hs],
                                            agg0t_ps[:, hs],
                                            dinv2_sb[:BS * F_IN, hs],
                                            ALU.mult)

        # ---- fused layer-1-weights / layer-2 stream --------------------
        # The 2048 (b,d) columns are processed as 4 groups of 512.  For
        # each group G the PE emits the 8 Z1 matmuls (one per node tile)
        # interleaved with the layer-2 work of group G-1 (aggregations +
        # Z2), so there is no phase barrier anywhere: while the PE runs
        # group G's Z1, ACT drains group G's act1 stream, the idle DMA
        # queues (SP + Pool SWDGE) move the aggregation results from PSUM
        # to SBUF, and ACT/DVE reduce the LeakyReLU pool.
        #
        #   z1  (m, G): [128, 512] fp8 DoubleRow matmul; act1 on ACT
        #   agg (c):    8 fp8 DoubleRow matmuls -> one [128, 1024] PSUM
        #   copy(c):    a2 = PSUM -> SBUF, plain DMA (dst dinv is in at2)
        #   z2  (c, n): [128, 512] fp32r matmul; LeakyReLU+pool into g
        #
        # PSUM budget: z1(2x1) + a2(2x2) + z2(2x1) = 8 banks exactly.
        with tc.tile_pool(name="ps_z1", bufs=2, space="PSUM") as ps_z1, \
             tc.tile_pool(name="ps_a2", bufs=2, space="PSUM") as ps_a2, \
             tc.tile_pool(name="ps_z2", bufs=2, space="PSUM") as ps_z2, \
             tc.tile_pool(name="sb_a2", bufs=3) as sb_a2, \
             tc.tile_pool(name="sb_h2", bufs=4) as sb_h2:

            def emit_z1(m, grp):
                cs = slice(grp * 512, (grp + 1) * 512)
                z1_ps = ps_z1.tile([128, 512], f32, tag="z1h")
                nc.tensor.matmul(z1_ps[:],
                                 agg0t_sb[:, :, m * 128:(m + 1) * 128],
                                 bd1_sb[:, :, cs],
                                 start=True, stop=not with_b1,
                                 perf_mode=DR)
                if with_b1:
                    nc.tensor.matmul(z1_ps[:], ones_sb[:, 0:128],
                                     b1_sb[:, cs], start=False, stop=True)
                nc.scalar.activation(act1_sb[:, m, cs], z1_ps[:], PRELU,
                                     alpha=NEG)

            dmaq = [0]

            def emit_agg(c):
                a2_ps = ps_a2.tile([128, N], f32, tag="a2ps")
                for kp in range(NT // 2):
                    for n in range(2):
                        nc.tensor.matmul(
                            a2_ps[:, n * 512:(n + 1) * 512],
                            act1_sb[:, 2 * kp:2 * kp + 2,
                                    c * 128:(c + 1) * 128],
                            at2_sb[:, 2 * kp:2 * kp + 2,
                                   n * 512:(n + 1) * 512],
                            start=(kp == 0), stop=(kp == NT // 2 - 1),
                            perf_mode=DR,
                        )
                a2_sb = sb_a2.tile([128, N], f32r, tag="a2sb")
                nc.vector.tensor_copy(a2_sb[:], a2_ps[:])
                return a2_sb

            h2_idx = [0]

            def emit_z2(c, n, a2_sb):
                z2_ps = ps_z2.tile([128, 512], f32, tag="z2h")
                nc.tensor.matmul(
                    z2_ps[:], bd2_sb[:],
                    a2_sb[:, n * 512:(n + 1) * 512],
                    start=True, stop=not with_b2)
                if with_b2:
                    nc.tensor.matmul(z2_ps[:], b2_sb[:],
                                     ones_sb[:, 0:512], start=False,
                                     stop=True)
                eng = H2_PATTERN[h2_idx[0] % len(H2_PATTERN)]
                h2_idx[0] += 1
                if eng == "act":
                    h2h = sb_h2.tile([128, 512], f8, tag="h2h")
                    nc.scalar.activation(h2h[:], z2_ps[:], PRELU, alpha=NEG,
                                         accum_out=gp_sb[:, n, c:c + 1])
                else:
                    # DVE cannot LeakyReLU out of PSUM in one pass; pool
                    # max(z,0) and min(z,0) separately (g recombined at
                    # the end).  Outputs are dead stores.
                    h2p = sb_h2.tile([128, 512], f8, tag="h2h")
                    nc.vector.tensor_scalar(
                        h2p[:], z2_ps[:], 0.0, 0.0, ALU.max, ALU.add,
                        accum_out=gp_sb[:, n, c:c + 1])
                    h2n = sb_h2.tile([128, 512], f8, tag="h2h")
                    nc.vector.tensor_scalar(
                        h2n[:], z2_ps[:], 0.0, 0.0, ALU.min, ALU.add,
                        accum_out=gn_sb[:, n, c:c + 1])

            # Per group: PE units for group G's z1 merge with the layer-2
            # PE units of group G-1; z2 lags its agg by two agg units so
            # the PE never waits on an in-flight PSUM->SBUF DMA (which
            # carries the ~900ns DMA-semaphore latency).
            pending = []   # (c, n, a2_sb) awaiting z2 emission

            def b_units(grp):
                c0 = grp * 4
                return [("agg", c0), ("agg", c0 + 1), ("agg", c0 + 2),
                        ("z2", None), ("z2", None), ("agg", c0 + 3),
                        ("z2", None), ("z2", None)]

            def run_b(unit):
                kind, c = unit
                if kind == "agg":
                    a2_sb = emit_agg(c)
                    pending.append((c, 0, a2_sb))
                    pending.append((c, 1, a2_sb))
                elif pending:
                    cc, n, a2_sb = pending.pop(0)
                    emit_z2(cc, n, a2_sb)

            for grp in range(4):
                bu = b_units(grp - 1) if grp >= 1 else []
                bi = 0
                for m in range(NT):
                    emit_z1(m, grp)
                    take = (len(bu) - bi + (NT - 1 - m)) // (NT - m)
                    for _ in range(take):
                        run_b(bu[bi])
                        bi += 1
            for unit in b_units(3):
                run_b(unit)
            while pending:
                cc, n, a2_sb = pending.pop(0)
                emit_z2(cc, n, a2_sb)

        # ---- projection: Y = LeakyReLU(G_slices.T @ Wp_s (+ bp)) ----
        # g = (sum LReLU parts) + NEG * (negative-side sums), then the two
        # 512-halves collapse.
        nc.vector.scalar_tensor_tensor(gf_sb[:], gn_sb[:], NEG, gp_sb[:],
                                       ALU.mult, ALU.add)
        nc.vector.tensor_tensor(gr_sb[:], gf_sb[:, 0, :], gf_sb[:, 1, :],
                                ALU.add)
        with tc.tile_pool(name="ps_y", bufs=2, space="PSUM") as ps_y:
            for half, out_sb in ((0, ye_sb), (1, yo_sb)):
                y_ps = ps_y.tile([BS // 2, D_OUT], f32, tag="yps")
                nc.tensor.matmul(y_ps[:], gr_sb[half * D2:(half + 1) * D2, :],
                                 wp_sb[half * D2:(half + 1) * D2, :],
                                 start=True, stop=not with_bp)
                if with_bp:
                    nc.tensor.matmul(
                        y_ps[:], ones_sb[:, 0:BS // 2],
                        bp_sb[:], start=False, stop=True)
                nc.scalar.activation(out_sb[:], y_ps[:], PRELU, alpha=NEG)

        y_r = y_t.rearrange("(c two) d -> two c d", two=2)
        nc.sync.dma_start(out=y_r[0, :, :], in_=ye_sb[:])
        nc.sync.dma_start(out=y_r[1, :, :], in_=yo_sb[:])


def kernel(x, edge_index, W1, b1, W2, b2, Wp, bp, _trace=False):
    x = np.ascontiguousarray(np.asarray(x, dtype=np.float32))
    consts = _build_host_constants(edge_index, W1, b1, W2, b2, Wp, bp)
    with_b1 = bool(np.any(consts["b1row"]))
    with_b2 = bool(np.any(consts["b2col"]))
    with_bp = bool(np.any(consts["bprow"]))

    nc = _build_program(with_b1, with_b2, with_bp)

    base = {"a01": consts["a01"], "a2d8": consts["a2d8"],
            "dinv2_bc": consts["dinv2_bc"], "bd1hl": consts["bd1hl"],
            "bd2": consts["bd2"], "wp_s": consts["wp_s"]}
    if with_b1:
        base["b1row"] = consts["b1row"]
    if with_b2:
        base["b2col"] = consts["b2col"]
    if with_bp:
        base["bprow"] = consts["bprow"]

    dinv = consts["dinv"]
    in_maps = [dict(base, x8=_prep_x8(x[c * BS:(c + 1) * BS], dinv))
               for c in range(NCORES)]

    from concourse.bass_utils import run_bass_kernel_spmd
    res = run_bass_kernel_spmd(nc, in_maps, core_ids=list(range(NCORES)),
                               trace=_trace)
    y = np.concatenate([res.results[c]["y"] for c in range(NCORES)], axis=0)
    out = np.ascontiguousarray(y.astype(np.float32))
    if _trace:
        return out, res
    return out


# revision 77
# speedup vs baseline: 1.0850x; 1.0850x over previous
"""GCN message-passing kernel for Trainium2 (8 NeuronCores, batch-parallel).

Model (see problem reference): two GCN layers over a fixed random graph
(N=1024 nodes, E=8192 directed edges, topology shared by all B=256
samples), LeakyReLU activations, global mean pool, Linear(64,128)+LeakyReLU.

Strategy
--------
Shared topology => GCN aggregation is a fixed linear operator per sample:
agg = A_hat @ h with A_hat = D^-1/2 (A+I) D^-1/2.  For layer 1, A_hat is
split into the *binary* matrix A+I (small integer counts - exactly
representable in fp8) and diagonal dinv scalings folded into neighbouring
ops (positive homogeneity of LeakyReLU lets dinv commute through the
activation); for layer 2 the dst-side dinv is folded into a second fp8
adjacency (~3% entry error that the mean pool averages away).  All
aggregation contractions and the Z1 weight matmul (via an fp8 hi+lo split
of BD1) run as fp8 DoubleRow matmuls on the PE: 2 k-tiles per pass at 0.5
cycles/col, ~4x the fp32r rate.

Data-parallel: 8 cores x 32 samples.  Per core, activations live in SBUF
as N-layout [node partition, (b, f) free] or T-layout [(b, f) partition,
node free]; feature matmuls use host-built block-diagonal weights so the
layouts flip for free between stages (zero explicit transposes):

  AGG0T[96,1024]   = sum_kp DR(H0'[kp], A01[kp])      (x' = dinv*x, fp8)
  AGG0T           *= dinv[dst]^2 -> fp8 x2             (DVE, PSUM->SBUF)
  Z1   [1024,2048] = DR(AGG0T_slices, BD1 hi+lo)       (fp8 DoubleRow)
  act1             = LeakyReLU(Z1) -> fp8              (ACT)
  AGG2T[2048,1024] = sum_kp DR(act1[kp], A01*dinv[kp]) (fp8 DoubleRow)
  a2               = PSUM -> SBUF copy                 (DVE)
  Z2T              = BD2.T @ a2                        (fp32r)
  G(pool)          = sum LeakyReLU(Z2T)                (ACT / DVE max+min)
  Y                = LeakyReLU(G_slices.T @ (Wp/1024)) (fp32r)

The layer-1-weight and layer-2 stages are fused into one software-
pipelined PE stream over four 512-column groups, with the elementwise
work balanced across ACT and DVE (GPSIMD cannot read PSUM on TRN2, and
the s2s2d2 ops cannot read PSUM twice, which pins LeakyReLU-from-PSUM to
ACT or a DVE max/min accumulation pair).
"""

import numpy as np

B = 256
N = 1024
F_IN = 3
D1 = 64
D2 = 64
D_OUT = 128
NEG = 0.01
NCORES = 8
BS = B // NCORES          # 32 samples per core
NT = N // 128             # 8 node tiles
BD1_COLS = BS * D1        # 2048
NCHUNK = BD1_COLS // 128  # 16 (b,d)-chunks of 128

# h2 engine rotation: the GPSIMD/Pool engine cannot read PSUM and the
# s2s2d2 STT op cannot read PSUM twice, so LeakyReLU straight out of
# PSUM runs either on ACT (Prelu, one pass) or on DVE as a max/min
# accumulation pair (g = sum max(z,0) + NEG * sum min(z,0)).
H2_PATTERN = ["act", "act", "dve"]


def _build_host_constants(edge_index, W1, b1, W2, b2, Wp, bp):
    """Binary adjacency (fp8-exact), dinv scalings, block-diag weights."""
    from concourse import mybir
    f8np = mybir.dt.np(mybir.dt.float8e4)

    src = np.asarray(edge_index[0], dtype=np.int64)
    dst = np.asarray(edge_index[1], dtype=np.int64)
    deg = np.bincount(dst, minlength=N).astype(np.float32) + 1.0
    dinv = (1.0 / np.sqrt(deg)).astype(np.float32)

    # a01[s, d] = #edges(s->d) + [s == d]; small ints, exact in fp8.
    a01 = np.zeros((N, N), dtype=np.float32)
    np.add.at(a01, (src, dst), 1.0)
    a01[np.arange(N), np.arange(N)] += 1.0
    a01_8 = np.ascontiguousarray(a01.astype(f8np))

    W1 = np.asarray(W1, dtype=np.float32)
    W2 = np.asarray(W2, dtype=np.float32)
    Wp = np.asarray(Wp, dtype=np.float32)

    # BD1[(b, f), (b, d)] = W1[f, d] on the block diagonal. [96, 2048]
    bd1 = np.zeros((BS * F_IN, BS * D1), dtype=np.float32)
    for b in range(BS):
        bd1[b * F_IN:(b + 1) * F_IN, b * D1:(b + 1) * D1] = W1
    # fp8 hi/lo split of BD1: hi + lo reproduces BD1 to ~0.4% so the Z1
    # matmul can run as one fp8 DoubleRow pass (contract hi and lo
    # k-copies in a single instruction at 0.5 cycles/col).
    bd1_hi = bd1.astype(f8np).astype(np.float32)
    bd1hl = np.stack([bd1_hi, bd1 - bd1_hi], axis=1)  # [96, 2, 2048]
    bd1hl_8 = np.ascontiguousarray(bd1hl.astype(f8np))
    # BD2 = blockdiag(W2, W2): one 128-row chunk covers 2 samples. [128, 128]
    bd2 = np.zeros((2 * D1, 2 * D2), dtype=np.float32)
    bd2[:D1, :D2] = W2
    bd2[D1:, D2:] = W2
    # Mean pool folded into the projection weight; stacked twice so both
    # halves of the pooled G tile have a matching rhs at the same base
    # partition.
    wp_s = np.vstack([Wp / float(N), Wp / float(N)]).astype(np.float32)

    consts = {
        "a01": a01_8,
        # layer-2 adjacency with the dst-side dinv folded in (fp8, ~3%
        # entry error; the mean pool averages it away)
        "a2d8": np.ascontiguousarray((a01 * dinv[None, :]).astype(f8np)),
        "dinv": dinv,
        "dinv2_bc": np.ascontiguousarray(
            np.broadcast_to((dinv * dinv)[None, :], (128, N))),
        "bd1hl": bd1hl_8,
        "bd2": bd2,
        "wp_s": wp_s,
        # bias rows (all zero for this problem; kept for generality)
        "b1row": np.tile(np.asarray(b1, np.float32), BS)[None, :],   # [1, 2048]
        "b2col": np.tile(np.asarray(b2, np.float32), 2)[None, :],    # [1, 128]
        "bprow": np.asarray(bp, np.float32)[None, :],                # [1, 128]
    }
    return consts


def _prep_x8(x_core, dinv):
    """x[b, 3n+f] -> fp8 tile [128, (nt b f)] pre-scaled by dinv[node]."""
    from concourse import mybir
    f8np = mybir.dt.np(mybir.dt.float8e4)
    xr = x_core.reshape(BS, N, F_IN) * dinv[None, :, None]
    x8 = xr.reshape(BS, NT, 128, F_IN).transpose(2, 1, 0, 3)
    return np.ascontiguousarray(x8.reshape(128, NT * BS * F_IN).astype(f8np))


_PROGRAM_CACHE = {}


def _build_program(with_b1, with_b2, with_bp, reps=1):
    key = (with_b1, with_b2, with_bp, reps)
    if key in _PROGRAM_CACHE:
        return _PROGRAM_CACHE[key]

    import concourse.mybir as mybir
    import concourse.tile as tile
    from concourse import bacc

    f32 = mybir.dt.float32
    f32r = mybir.dt.float32r
    f8 = mybir.dt.float8e4

    # Bacc (not raw Bass): its compile() runs move_matmul_waits_to_ldweights
    # + generate_event_semaphores, which split sync waits down to the 1-per-
    # instruction hardware limit (self-loading fp32r matmuls hit this).
    nc = bacc.Bacc(trn_type="TRN2", target_bir_lowering=False, debug=False)

    x8_t = nc.dram_tensor("x8", [128, NT * BS * F_IN], f8,
                          kind="ExternalInput").ap()
    at_t = nc.dram_tensor("a01", [N, N], f8, kind="ExternalInput").ap()
    dbc_t = nc.dram_tensor("a2d8", [N, N], f8, kind="ExternalInput").ap()
    dcol_t = nc.dram_tensor("dinv2_bc", [128, N], f32,
                            kind="ExternalInput").ap()
    bd1_t = nc.dram_tensor("bd1hl", [BS * F_IN, 2, BS * D1], f8,
                           kind="ExternalInput").ap()
    bd2_t = nc.dram_tensor("bd2", [128, 128], f32r,
                           kind="ExternalInput").ap()
    wp_t = nc.dram_tensor("wp_s", [2 * D2, D_OUT], f32r,
                          kind="ExternalInput").ap()
    b1_t = nc.dram_tensor("b1row", [1, BS * D1], f32r,
                          kind="ExternalInput").ap() if with_b1 else None
    b2_t = nc.dram_tensor("b2col", [1, 128], f32r,
                          kind="ExternalInput").ap() if with_b2 else None
    bp_t = nc.dram_tensor("bprow", [1, D_OUT], f32r,
                          kind="ExternalInput").ap() if with_bp else None
    y_t = nc.dram_tensor("y", [BS, D_OUT], f32, kind="ExternalOutput").ap()

    tensors = (x8_t, at_t, dbc_t, dcol_t, bd1_t, bd2_t, wp_t,
               b1_t, b2_t, bp_t, y_t)

    with tile.TileContext(nc) as tc:
        if reps > 1:
            with tc.For_i(0, reps, 1):
                _emit_body(nc, tc, mybir, tensors, with_b1, with_b2, with_bp)
        else:
            _emit_body(nc, tc, mybir, tensors, with_b1, with_b2, with_bp)

    nc.compile()
    _PROGRAM_CACHE[key] = nc
    return nc


def _emit_body(nc, tc, mybir, tensors, with_b1, with_b2, with_bp):
    from contextlib import ExitStack

    (x8_t, at_t, dbc_t, dcol_t, bd1_t, bd2_t, wp_t,
     b1_t, b2_t, bp_t, y_t) = tensors

    f32 = mybir.dt.float32
    f32r = mybir.dt.float32r
    f8 = mybir.dt.float8e4
    DR = mybir.MatmulPerfMode.DoubleRow
    ALU = mybir.AluOpType
    PRELU = mybir.ActivationFunctionType.Prelu
    ENG = {"act": nc.scalar, "dve": nc.vector, "pool": nc.gpsimd}

    with ExitStack() as es:
        const = es.enter_context(tc.tile_pool(name="const", bufs=1))
        work = es.enter_context(tc.tile_pool(name="work", bufs=1))

        at_sb = const.tile([128, NT, N], f8)        # A01 [src-part, kt, dst]
        at2_sb = const.tile([128, NT, N], f8)       # A01 * dinv[dst], fp8
        dinv2_sb = const.tile([128, N], f32)        # dinv^2 bcast over parts
        bd1_sb = const.tile([BS * F_IN, 2, BS * D1], f8)   # hi/lo pair
        bd2_sb = const.tile([128, 128], f32r)
        wp_sb = const.tile([2 * D2, D_OUT], f32r)
        h0_sb = work.tile([128, NT, BS, F_IN], f8)  # dinv*x as [node, (b,f)]
        agg0t_sb = work.tile([BS * F_IN, 2, N], f8)  # two k-copies for DR
        act1_sb = work.tile([128, NT, BD1_COLS], f8)
        gp_sb = work.tile([128, 2, NCHUNK], f32)   # sum LReLU | sum max(z,0)
        gn_sb = work.tile([128, 2, NCHUNK], f32)   # sum min(z,0) (DVE path)
        gf_sb = work.tile([128, 2, NCHUNK], f32)
        gr_sb = work.tile([128, NCHUNK], f32r)
        ye_sb = work.tile([BS // 2, D_OUT], f32)
        yo_sb = work.tile([BS // 2, D_OUT], f32)
        if with_b1:
            b1_sb = const.tile([1, BS * D1], f32r)
        if with_b2:
            b2_sb = const.tile([1, 128], f32r)
        if with_bp:
            bp_sb = const.tile([1, D_OUT], f32r)
        if with_b1 or with_b2 or with_bp:
            ones_f = const.tile([1, 512], f32)
            ones_sb = const.tile([1, 512], f32r)
            nc.any.memset(ones_f[:], 1.0)
            nc.vector.tensor_copy(ones_sb[:], ones_f[:])

        # ---- loads: everything DMAs straight into its operand tile.
        # SP feeds the PE-critical stream in consumption order (x8, A01
        # tiles, then bd1 group slices); ACT's queue brings the dinv rows
        # and small weights before its first elementwise piece is due.
        # Pool/DVE queues stay free for elementwise work. ----
        x8_r = x8_t.rearrange("p (kt b f) -> p kt b f", kt=NT, b=BS, f=F_IN)
        nc.sync.dma_start(out=h0_sb[:], in_=x8_r)
        at_r = at_t.rearrange("(kt p) d -> p kt d", kt=NT, p=128)
        for k in range(NT):
            nc.sync.dma_start(out=at_sb[:, k, :], in_=at_r[:, k, :])
        for grp in range(4):
            cs = slice(grp * 512, (grp + 1) * 512)
            nc.sync.dma_start(out=bd1_sb[:, :, cs], in_=bd1_t[:, :, cs])
        nc.gpsimd.dma_start(out=dinv2_sb[:], in_=dcol_t)
        at2_r = dbc_t.rearrange("(kt p) d -> p kt d", kt=NT, p=128)
        for k in range(NT):
            nc.gpsimd.dma_start(out=at2_sb[:, k, :], in_=at2_r[:, k, :])
        nc.gpsimd.dma_start(out=bd2_sb[:], in_=bd2_t)
        nc.gpsimd.dma_start(out=wp_sb[:], in_=wp_t)
        if with_b1:
            nc.scalar.dma_start(out=b1_sb[:], in_=b1_t)
        if with_b2:
            nc.scalar.dma_start(out=b2_sb[:], in_=b2_t)
        if with_bp:
            nc.scalar.dma_start(out=bp_sb[:], in_=bp_t)
        # g2's DVE min-accumulation slots must start from zero for the
        # chunks whose LeakyReLU runs fully on ACT (they never write gN).
        nc.vector.memset(gn_sb[:], 0.0)

        # ---- layer 1 aggregation: AGG0T = sum_kp H0'[kp].T @ A01[kp],
        #      then *= dinv[dst] on the PSUM->SBUF copy ----
        with tc.tile_pool(name="ps_agg0", bufs=1, space="PSUM") as ps_agg0:
            agg0t_ps = ps_agg0.tile([BS * F_IN, N], f32)
            for kp in range(NT // 2):
                for n in range(2):
                    nc.tensor.matmul(
                        agg0t_ps[:, n * 512:(n + 1) * 512],
                        h0_sb[:, 2 * kp:2 * kp + 2, :, :],
                        at_sb[:, 2 * kp:2 * kp + 2, n * 512:(n + 1) * 512],
                        start=(kp == 0), stop=(kp == NT // 2 - 1),
                        perf_mode=DR,
                    )
            # dinv^2: one dinv is layer 1's dst scaling, the other
            # premultiplies layer 2's source side (pulled through the
            # LeakyReLU by positive homogeneity), so act1 below needs no
            # per-partition scale and can run on any engine.  Written
            # twice (fp8) so the Z1 DoubleRow matmul sees the two
            # k-copies its hi/lo weight split contracts against; split in
            # halves so the first z1 matmul unblocks sooner.
            # half-outer order: the first two ops cover both k-copies of
            # node tiles 0..3, unblocking z1(0) as early as possible
            for half in range(2):
                hs = slice(half * 512, (half + 1) * 512)
                for j in range(2):
                    nc.vector.tensor_tensor(agg0t_sb[:, j, hs],
                                            agg0t_ps[:, hs],
                                            dinv2_sb[:BS * F_IN, hs],
                                            ALU.mult)

        # ---- fused layer-1-weights / layer-2 stream --------------------
        # The 2048 (b,d) columns are processed as 4 groups of 512.  For
        # each group G the PE emits the 8 Z1 matmuls (one per node tile)
        # interleaved with the layer-2 work of group G-1 (aggregations +
        # Z2), so there is no phase barrier anywhere: while the PE runs
        # group G's Z1, ACT drains group G's act1 stream, the idle DMA
        # queues (SP + Pool SWDGE) move the aggregation results from PSUM
        # to SBUF, and ACT/DVE reduce the LeakyReLU pool.
        #
        #   z1  (m, G): [128, 512] fp8 DoubleRow matmul; act1 on ACT
        #   agg (c):    8 fp8 DoubleRow matmuls -> one [128, 1024] PSUM
        #   copy(c):    a2 = PSUM -> SBUF, plain DMA (dst dinv is in at2)
        #   z2  (c, n): [128, 512] fp32r matmul; LeakyReLU+pool into g
        #
        # PSUM budget: z1(2x1) + a2(2x2) + z2(2x1) = 8 banks exactly.
        # z1 and z2 tiles share one 4-deep PSUM ring: same shape, and the
        # shared rotation lets the PE run further ahead of the ACT/DVE
        # drain than 2 dedicated buffers each would.
        with tc.tile_pool(name="ps_small", bufs=4, space="PSUM") as ps_small, \
             tc.tile_pool(name="ps_a2", bufs=2, space="PSUM") as ps_a2, \
             tc.tile_pool(name="sb_a2", bufs=4) as sb_a2, \
             tc.tile_pool(name="sb_h2", bufs=6) as sb_h2:

            def emit_z1(m, grp):
                cs = slice(grp * 512, (grp + 1) * 512)
                z1_ps = ps_small.tile([128, 512], f32, tag="psmall")
                nc.tensor.matmul(z1_ps[:],
                                 agg0t_sb[:, :, m * 128:(m + 1) * 128],
                                 bd1_sb[:, :, cs],
                                 start=True, stop=not with_b1,
                                 perf_mode=DR)
                if with_b1:
                    nc.tensor.matmul(z1_ps[:], ones_sb[:, 0:128],
                                     b1_sb[:, cs], start=False, stop=True)
                nc.scalar.activation(act1_sb[:, m, cs], z1_ps[:], PRELU,
                                     alpha=NEG)

            dmaq = [0]

            def emit_agg(c):
                a2_ps = ps_a2.tile([128, N], f32, tag="a2ps")
                a2_sb = sb_a2.tile([128, N], f32r, tag="a2sb")
                # half-major order: half 0's PSUM->SBUF copy overlaps the
                # PE's half-1 matmuls, shortening the agg->z2 chain
                for n in range(2):
                    cs = slice(n * 512, (n + 1) * 512)
                    for kp in range(NT // 2):
                        nc.tensor.matmul(
                            a2_ps[:, cs],
                            act1_sb[:, 2 * kp:2 * kp + 2,
                                    c * 128:(c + 1) * 128],
                            at2_sb[:, 2 * kp:2 * kp + 2, cs],
                            start=(kp == 0), stop=(kp == NT // 2 - 1),
                            perf_mode=DR,
                        )
                    nc.vector.tensor_copy(a2_sb[:, cs], a2_ps[:, cs])
                return a2_sb

            h2_idx = [0]
            h2_queue = []   # (c, n, z2_ps) with h2 emission lagged one unit

            def emit_z2(c, n, a2_sb):
                z2_ps = ps_small.tile([128, 512], f32, tag="psmall")
                nc.tensor.matmul(
                    z2_ps[:], bd2_sb[:],
                    a2_sb[:, n * 512:(n + 1) * 512],
                    start=True, stop=not with_b2)
                if with_b2:
                    nc.tensor.matmul(z2_ps[:], b2_sb[:],
                                     ones_sb[:, 0:512], start=False,
                                     stop=True)
                h2_queue.append((c, n, z2_ps))
                if len(h2_queue) > 1:
                    emit_h2(*h2_queue.pop(0))

            def emit_h2(c, n, z2_ps):
                eng = H2_PATTERN[h2_idx[0] % len(H2_PATTERN)]
                h2_idx[0] += 1
                if eng == "act":
                    h2h = sb_h2.tile([128, 512], f8, tag="h2h")
                    nc.scalar.activation(h2h[:], z2_ps[:], PRELU, alpha=NEG,
                                         accum_out=gp_sb[:, n, c:c + 1])
                else:
                    # DVE cannot LeakyReLU out of PSUM in one pass; pool
                    # max(z,0) and min(z,0) separately (g recombined at
                    # the end).  Outputs are dead stores.
                    h2p = sb_h2.tile([128, 512], f8, tag="h2h")
                    nc.vector.tensor_scalar(
                        h2p[:], z2_ps[:], 0.0, 0.0, ALU.max, ALU.add,
                        accum_out=gp_sb[:, n, c:c + 1])
                    h2n = sb_h2.tile([128, 512], f8, tag="h2h")
                    nc.vector.tensor_scalar(
                        h2n[:], z2_ps[:], 0.0, 0.0, ALU.min, ALU.add,
                        accum_out=gn_sb[:, n, c:c + 1])

            # Per group: PE units for group G's z1 merge with the layer-2
            # PE units of group G-1; z2 lags its agg by two agg units so
            # the PE never waits on an in-flight PSUM->SBUF DMA (which
            # carries the ~900ns DMA-semaphore latency).
            pending = []   # (c, n, a2_sb) awaiting z2 emission

            def b_units(grp):
                c0 = grp * 4
                return [("agg", c0), ("agg", c0 + 1), ("agg", c0 + 2),
                        ("z2", None), ("z2", None), ("agg", c0 + 3),
                        ("z2", None), ("z2", None)]

            def run_b(unit):
                kind, c = unit
                if kind == "agg":
                    a2_sb = emit_agg(c)
                    pending.append((c, 0, a2_sb))
                    pending.append((c, 1, a2_sb))
                elif pending:
                    cc, n, a2_sb = pending.pop(0)
                    emit_z2(cc, n, a2_sb)

            for grp in range(4):
                bu = b_units(grp - 1) if grp >= 1 else []
                bi = 0
                for m in range(NT):
                    emit_z1(m, grp)
                    take = (len(bu) - bi + (NT - 1 - m)) // (NT - m)
                    for _ in range(take):
                        run_b(bu[bi])
                        bi += 1
            for unit in b_units(3):
                run_b(unit)
            while pending:
                cc, n, a2_sb = pending.pop(0)
                emit_z2(cc, n, a2_sb)
            while h2_queue:
                emit_h2(*h2_queue.pop(0))

        # ---- projection: Y = LeakyReLU(G_slices.T @ Wp_s (+ bp)) ----
        # g = (sum LReLU parts) + NEG * (negative-side sums), then the two
        # 512-halves collapse.
        nc.vector.scalar_tensor_tensor(gf_sb[:], gn_sb[:], NEG, gp_sb[:],
                                       ALU.mult, ALU.add)
        nc.vector.tensor_tensor(gr_sb[:], gf_sb[:, 0, :], gf_sb[:, 1, :],
                                ALU.add)
        with tc.tile_pool(name="ps_y", bufs=2, space="PSUM") as ps_y:
            for half, out_sb in ((0, ye_sb), (1, yo_sb)):
                y_ps = ps_y.tile([BS // 2, D_OUT], f32, tag="yps")
                nc.tensor.matmul(y_ps[:], gr_sb[half * D2:(half + 1) * D2, :],
                                 wp_sb[half * D2:(half + 1) * D2, :],
                                 start=True, stop=not with_bp)
                if with_bp:
                    nc.tensor.matmul(
                        y_ps[:], ones_sb[:, 0:BS // 2],
                        bp_sb[:], start=False, stop=True)
                nc.scalar.activation(out_sb[:], y_ps[:], PRELU, alpha=NEG)

        y_r = y_t.rearrange("(c two) d -> two c d", two=2)
        nc.sync.dma_start(out=y_r[0, :, :], in_=ye_sb[:])
        nc.sync.dma_start(out=y_r[1, :, :], in_=yo_sb[:])


def kernel(x, edge_index, W1, b1, W2, b2, Wp, bp, _trace=False):
    x = np.ascontiguousarray(np.asarray(x, dtype=np.float32))
    consts = _build_host_constants(edge_index, W1, b1, W2, b2, Wp, bp)
    with_b1 = bool(np.any(consts["b1row"]))
    with_b2 = bool(np.any(consts["b2col"]))
    with_bp = bool(np.any(consts["bprow"]))

    nc = _build_program(with_b1, with_b2, with_bp)

    base = {"a01": consts["a01"], "a2d8": consts["a2d8"],
            "dinv2_bc": consts["dinv2_bc"], "bd1hl": consts["bd1hl"],
            "bd2": consts["bd2"], "wp_s": consts["wp_s"]}
    if with_b1:
        base["b1row"] = consts["b1row"]
    if with_b2:
        base["b2col"] = consts["b2col"]
    if with_bp:
        base["bprow"] = consts["bprow"]

    dinv = consts["dinv"]
    in_maps = [dict(base, x8=_prep_x8(x[c * BS:(c + 1) * BS], dinv))
               for c in range(NCORES)]

    from concourse.bass_utils import run_bass_kernel_spmd
    res = run_bass_kernel_spmd(nc, in_maps, core_ids=list(range(NCORES)),
                               trace=_trace)
    y = np.concatenate([res.results[c]["y"] for c in range(NCORES)], axis=0)
    out = np.ascontiguousarray(y.astype(np.float32))
    if _trace:
        return out, res
    return out


# revision 86
# speedup vs baseline: 1.2360x; 1.1392x over previous
"""GCN message-passing kernel for Trainium2 (8 NeuronCores, batch-parallel).

Model (see problem reference): two GCN layers over a fixed random graph
(N=1024 nodes, E=8192 directed edges, topology shared by all B=256
samples), LeakyReLU activations, global mean pool, Linear(64,128)+LeakyReLU.

Strategy
--------
Shared topology => GCN aggregation is a fixed linear operator per sample:
agg = A_hat @ h with A_hat = D^-1/2 (A+I) D^-1/2.  For layer 1, A_hat is
split into the *binary* matrix A+I (small integer counts - exactly
representable in fp8) and diagonal dinv scalings folded into neighbouring
ops (positive homogeneity of LeakyReLU lets dinv commute through the
activation); for layer 2 the dst-side dinv is folded into a second fp8
adjacency (~3% entry error that the mean pool averages away).  All
aggregation contractions and the Z1 weight matmul (via an fp8 hi+lo split
of BD1) run as fp8 DoubleRow matmuls on the PE: 2 k-tiles per pass at 0.5
cycles/col, ~4x the fp32r rate.

Data-parallel: 8 cores x 32 samples.  Per core, activations live in SBUF
as N-layout [node partition, (b, f) free] or T-layout [(b, f) partition,
node free]; feature matmuls use host-built block-diagonal weights so the
layouts flip for free between stages (zero explicit transposes):

  AGG0T[96,1024]   = sum_kp DR(H0'[kp], A01[kp])      (x' = dinv*x, fp8)
  AGG0T           *= dinv[dst]^2 -> fp8 x2             (DVE, PSUM->SBUF)
  Z1   [1024,2048] = DR(AGG0T_slices, BD1 hi+lo)       (fp8 DoubleRow)
  act1             = LeakyReLU(Z1) -> fp8              (ACT)
  AGG2T[2048,1024] = sum_kp DR(act1[kp], A01*dinv[kp]) (fp8 DoubleRow)
  a2               = PSUM -> SBUF copy                 (DVE)
  Z2T              = BD2.T @ a2                        (fp32r)
  G(pool)          = sum LeakyReLU(Z2T)                (ACT / DVE max+min)
  Y                = LeakyReLU(G_slices.T @ (Wp/1024)) (fp32r)

The layer-1-weight and layer-2 stages are fused into one software-
pipelined PE stream over four 512-column groups, with the elementwise
work balanced across ACT and DVE (GPSIMD cannot read PSUM on TRN2, and
the s2s2d2 ops cannot read PSUM twice, which pins LeakyReLU-from-PSUM to
ACT or a DVE max/min accumulation pair).
"""

import numpy as np

B = 256
N = 1024
F_IN = 3
D1 = 64
D2 = 64
D_OUT = 128
NEG = 0.01
NCORES = 8
BS = B // NCORES          # 32 samples per core
NT = N // 128             # 8 node tiles
BD1_COLS = BS * D1        # 2048
NCHUNK = BD1_COLS // 128  # 16 (b,d)-chunks of 128

# Engine rotation for the layer-2 copy/pool ops.  Via
#   sum LReLU(z) = (1-NEG) * sum max(z,0) + NEG * sum z
# and sum_dst z2 = BD2.T @ (sum_dst a2)  (a free accum on the a2 copies
# plus one tiny matmul), the pooled LeakyReLU needs only a single-pass
# max+accum, which ACT (Relu activation) and DVE (tensor_scalar max)
# can both run; act1 stays ACT-pinned, so DVE takes most of these.
L2_PATTERN = ["dve", "dve", "dve", "act"]


def _build_host_constants(edge_index, W1, b1, W2, b2, Wp, bp):
    """Binary adjacency (fp8-exact), dinv scalings, block-diag weights."""
    from concourse import mybir
    f8np = mybir.dt.np(mybir.dt.float8e4)

    src = np.asarray(edge_index[0], dtype=np.int64)
    dst = np.asarray(edge_index[1], dtype=np.int64)
    deg = np.bincount(dst, minlength=N).astype(np.float32) + 1.0
    dinv = (1.0 / np.sqrt(deg)).astype(np.float32)

    # a01[s, d] = #edges(s->d) + [s == d]; small ints, exact in fp8.
    a01 = np.zeros((N, N), dtype=np.float32)
    np.add.at(a01, (src, dst), 1.0)
    a01[np.arange(N), np.arange(N)] += 1.0
    a01_8 = np.ascontiguousarray(a01.astype(f8np))

    W1 = np.asarray(W1, dtype=np.float32)
    W2 = np.asarray(W2, dtype=np.float32)
    Wp = np.asarray(Wp, dtype=np.float32)

    # BD1[(b, f), (b, d)] = W1[f, d] on the block diagonal. [96, 2048]
    bd1 = np.zeros((BS * F_IN, BS * D1), dtype=np.float32)
    for b in range(BS):
        bd1[b * F_IN:(b + 1) * F_IN, b * D1:(b + 1) * D1] = W1
    # fp8 hi/lo split of BD1: hi + lo reproduces BD1 to ~0.4% so the Z1
    # matmul can run as one fp8 DoubleRow pass (contract hi and lo
    # k-copies in a single instruction at 0.5 cycles/col).
    bd1_hi = bd1.astype(f8np).astype(np.float32)
    bd1hl = np.stack([bd1_hi, bd1 - bd1_hi], axis=1)  # [96, 2, 2048]
    bd1hl_8 = np.ascontiguousarray(bd1hl.astype(f8np))
    # BD2 = blockdiag(W2, W2): one 128-row chunk covers 2 samples. [128, 128]
    bd2 = np.zeros((2 * D1, 2 * D2), dtype=np.float32)
    bd2[:D1, :D2] = W2
    bd2[D1:, D2:] = W2
    # Mean pool folded into the projection weight; stacked twice so both
    # halves of the pooled G tile have a matching rhs at the same base
    # partition.
    wp_s = np.vstack([Wp / float(N), Wp / float(N)]).astype(np.float32)

    consts = {
        "a01": a01_8,
        # layer-2 adjacency with the dst-side dinv folded in (fp8, ~3%
        # entry error; the mean pool averages it away)
        "a2d8": np.ascontiguousarray((a01 * dinv[None, :]).astype(f8np)),
        "dinv": dinv,
        "dinv2_bc": np.ascontiguousarray(
            np.broadcast_to((dinv * dinv)[None, :], (128, N))),
        "bd1hl": bd1hl_8,
        "bd2": bd2,
        "wp_s": wp_s,
        # bias rows (all zero for this problem; kept for generality)
        "b1row": np.tile(np.asarray(b1, np.float32), BS)[None, :],   # [1, 2048]
        "b2col": np.tile(np.asarray(b2, np.float32), 2)[None, :],    # [1, 128]
        "bprow": np.asarray(bp, np.float32)[None, :],                # [1, 128]
    }
    return consts


def _prep_x8(x_core, dinv):
    """x[b, 3n+f] -> fp8 tile [128, (nt b f)] pre-scaled by dinv[node]."""
    from concourse import mybir
    f8np = mybir.dt.np(mybir.dt.float8e4)
    xr = x_core.reshape(BS, N, F_IN) * dinv[None, :, None]
    x8 = xr.reshape(BS, NT, 128, F_IN).transpose(2, 1, 0, 3)
    return np.ascontiguousarray(x8.reshape(128, NT * BS * F_IN).astype(f8np))


_PROGRAM_CACHE = {}


def _build_program(with_b1, with_b2, with_bp, reps=1):
    key = (with_b1, with_b2, with_bp, reps)
    if key in _PROGRAM_CACHE:
        return _PROGRAM_CACHE[key]

    import concourse.mybir as mybir
    import concourse.tile as tile
    from concourse import bacc

    f32 = mybir.dt.float32
    f32r = mybir.dt.float32r
    f8 = mybir.dt.float8e4

    # Bacc (not raw Bass): its compile() runs move_matmul_waits_to_ldweights
    # + generate_event_semaphores, which split sync waits down to the 1-per-
    # instruction hardware limit (self-loading fp32r matmuls hit this).
    nc = bacc.Bacc(trn_type="TRN2", target_bir_lowering=False, debug=False)

    x8_t = nc.dram_tensor("x8", [128, NT * BS * F_IN], f8,
                          kind="ExternalInput").ap()
    at_t = nc.dram_tensor("a01", [N, N], f8, kind="ExternalInput").ap()
    dbc_t = nc.dram_tensor("a2d8", [N, N], f8, kind="ExternalInput").ap()
    dcol_t = nc.dram_tensor("dinv2_bc", [128, N], f32,
                            kind="ExternalInput").ap()
    bd1_t = nc.dram_tensor("bd1hl", [BS * F_IN, 2, BS * D1], f8,
                           kind="ExternalInput").ap()
    bd2_t = nc.dram_tensor("bd2", [128, 128], f32r,
                           kind="ExternalInput").ap()
    wp_t = nc.dram_tensor("wp_s", [2 * D2, D_OUT], f32r,
                          kind="ExternalInput").ap()
    b1_t = nc.dram_tensor("b1row", [1, BS * D1], f32r,
                          kind="ExternalInput").ap() if with_b1 else None
    b2_t = nc.dram_tensor("b2col", [1, 128], f32r,
                          kind="ExternalInput").ap() if with_b2 else None
    bp_t = nc.dram_tensor("bprow", [1, D_OUT], f32r,
                          kind="ExternalInput").ap() if with_bp else None
    y_t = nc.dram_tensor("y", [BS, D_OUT], f32, kind="ExternalOutput").ap()

    tensors = (x8_t, at_t, dbc_t, dcol_t, bd1_t, bd2_t, wp_t,
               b1_t, b2_t, bp_t, y_t)

    with tile.TileContext(nc) as tc:
        if reps > 1:
            with tc.For_i(0, reps, 1):
                _emit_body(nc, tc, mybir, tensors, with_b1, with_b2, with_bp)
        else:
            _emit_body(nc, tc, mybir, tensors, with_b1, with_b2, with_bp)

    nc.compile()
    _PROGRAM_CACHE[key] = nc
    return nc


def _emit_body(nc, tc, mybir, tensors, with_b1, with_b2, with_bp):
    from contextlib import ExitStack

    (x8_t, at_t, dbc_t, dcol_t, bd1_t, bd2_t, wp_t,
     b1_t, b2_t, bp_t, y_t) = tensors

    f32 = mybir.dt.float32
    f32r = mybir.dt.float32r
    f8 = mybir.dt.float8e4
    DR = mybir.MatmulPerfMode.DoubleRow
    ALU = mybir.AluOpType
    PRELU = mybir.ActivationFunctionType.Prelu
    ENG = {"act": nc.scalar, "dve": nc.vector, "pool": nc.gpsimd}

    with ExitStack() as es:
        const = es.enter_context(tc.tile_pool(name="const", bufs=1))
        work = es.enter_context(tc.tile_pool(name="work", bufs=1))

        at_sb = const.tile([128, NT, N], f8)        # A01 [src-part, kt, dst]
        at2_sb = const.tile([128, NT, N], f8)       # A01 * dinv[dst], fp8
        dinv2_sb = const.tile([128, N], f32)        # dinv^2 bcast over parts
        bd1_sb = const.tile([BS * F_IN, 2, BS * D1], f8)   # hi/lo pair
        bd2_sb = const.tile([128, 128], f32r)
        wp_sb = const.tile([2 * D2, D_OUT], f32r)
        h0_sb = work.tile([128, NT, BS, F_IN], f8)  # dinv*x as [node, (b,f)]
        agg0t_sb = work.tile([BS * F_IN, 2, N], f8)  # two k-copies for DR
        act1_sb = work.tile([128, NT, BD1_COLS], f8)
        gp_sb = work.tile([128, 2, NCHUNK], f32)   # sum max(z2, 0)
        s_sb = work.tile([128, 2, NCHUNK], f32)    # sum_dst a2 (copy accums)
        zs_sb = work.tile([128, 2, NCHUNK], f32)   # sum_dst z2 = BD2.T @ s
        gf_sb = work.tile([128, 2, NCHUNK], f32)
        gr_sb = work.tile([128, NCHUNK], f32r)
        ye_sb = work.tile([BS // 2, D_OUT], f32)
        yo_sb = work.tile([BS // 2, D_OUT], f32)
        if with_b1:
            b1_sb = const.tile([1, BS * D1], f32r)
        if with_b2:
            b2_sb = const.tile([1, 128], f32r)
        if with_bp:
            bp_sb = const.tile([1, D_OUT], f32r)
        if with_b1 or with_b2 or with_bp:
            ones_f = const.tile([1, 512], f32)
            ones_sb = const.tile([1, 512], f32r)
            nc.any.memset(ones_f[:], 1.0)
            nc.vector.tensor_copy(ones_sb[:], ones_f[:])

        # ---- loads: everything DMAs straight into its operand tile.
        # SP feeds the PE-critical stream in consumption order (x8, A01
        # tiles, then bd1 group slices); ACT's queue brings the dinv rows
        # and small weights before its first elementwise piece is due.
        # Pool/DVE queues stay free for elementwise work. ----
        x8_r = x8_t.rearrange("p (kt b f) -> p kt b f", kt=NT, b=BS, f=F_IN)
        nc.sync.dma_start(out=h0_sb[:], in_=x8_r)
        at_r = at_t.rearrange("(kt p) d -> p kt d", kt=NT, p=128)
        for k in range(NT):
            nc.sync.dma_start(out=at_sb[:, k, :], in_=at_r[:, k, :])
        for grp in range(4):
            cs = slice(grp * 512, (grp + 1) * 512)
            nc.sync.dma_start(out=bd1_sb[:, :, cs], in_=bd1_t[:, :, cs])
        nc.gpsimd.dma_start(out=dinv2_sb[:], in_=dcol_t)
        at2_r = dbc_t.rearrange("(kt p) d -> p kt d", kt=NT, p=128)
        for k in range(NT):
            nc.gpsimd.dma_start(out=at2_sb[:, k, :], in_=at2_r[:, k, :])
        nc.gpsimd.dma_start(out=bd2_sb[:], in_=bd2_t)
        nc.gpsimd.dma_start(out=wp_sb[:], in_=wp_t)
        if with_b1:
            nc.scalar.dma_start(out=b1_sb[:], in_=b1_t)
        if with_b2:
            nc.scalar.dma_start(out=b2_sb[:], in_=b2_t)
        if with_bp:
            nc.scalar.dma_start(out=bp_sb[:], in_=bp_t)


        # ---- layer 1 aggregation: AGG0T = sum_kp H0'[kp].T @ A01[kp],
        #      then *= dinv[dst] on the PSUM->SBUF copy ----
        with tc.tile_pool(name="ps_agg0", bufs=1, space="PSUM") as ps_agg0:
            agg0t_ps = ps_agg0.tile([BS * F_IN, N], f32)
            for kp in range(NT // 2):
                for n in range(2):
                    nc.tensor.matmul(
                        agg0t_ps[:, n * 512:(n + 1) * 512],
                        h0_sb[:, 2 * kp:2 * kp + 2, :, :],
                        at_sb[:, 2 * kp:2 * kp + 2, n * 512:(n + 1) * 512],
                        start=(kp == 0), stop=(kp == NT // 2 - 1),
                        perf_mode=DR,
                    )
            # dinv^2: one dinv is layer 1's dst scaling, the other
            # premultiplies layer 2's source side (pulled through the
            # LeakyReLU by positive homogeneity), so act1 below needs no
            # per-partition scale and can run on any engine.  Written
            # twice (fp8) so the Z1 DoubleRow matmul sees the two
            # k-copies its hi/lo weight split contracts against; split in
            # halves so the first z1 matmul unblocks sooner.
            # half-outer order: the first two ops cover both k-copies of
            # node tiles 0..3, unblocking z1(0) as early as possible
            for half in range(2):
                hs = slice(half * 512, (half + 1) * 512)
                for j in range(2):
                    nc.vector.tensor_tensor(agg0t_sb[:, j, hs],
                                            agg0t_ps[:, hs],
                                            dinv2_sb[:BS * F_IN, hs],
                                            ALU.mult)

        # ---- fused layer-1-weights / layer-2 stream --------------------
        # The 2048 (b,d) columns are processed as 4 groups of 512.  For
        # each group G the PE emits the 8 Z1 matmuls (one per node tile)
        # interleaved with the layer-2 work of group G-1 (aggregations +
        # Z2), so there is no phase barrier anywhere: while the PE runs
        # group G's Z1, ACT drains group G's act1 stream, the idle DMA
        # queues (SP + Pool SWDGE) move the aggregation results from PSUM
        # to SBUF, and ACT/DVE reduce the LeakyReLU pool.
        #
        #   z1  (m, G): [128, 512] fp8 DoubleRow matmul; act1 on ACT
        #   agg (c):    8 fp8 DoubleRow matmuls -> one [128, 1024] PSUM
        #   copy(c):    a2 = PSUM -> SBUF, plain DMA (dst dinv is in at2)
        #   z2  (c, n): [128, 512] fp32r matmul; LeakyReLU+pool into g
        #
        # PSUM budget: z1(2x1) + a2(2x2) + z2(2x1) = 8 banks exactly.
        # z1 and z2 tiles share one 4-deep PSUM ring: same shape, and the
        # shared rotation lets the PE run further ahead of the ACT/DVE
        # drain than 2 dedicated buffers each would.
        with tc.tile_pool(name="ps_small", bufs=4, space="PSUM") as ps_small, \
             tc.tile_pool(name="ps_a2", bufs=2, space="PSUM") as ps_a2, \
             tc.tile_pool(name="sb_a2", bufs=4) as sb_a2, \
             tc.tile_pool(name="sb_h2", bufs=6) as sb_h2:

            def emit_z1(m, grp):
                cs = slice(grp * 512, (grp + 1) * 512)
                z1_ps = ps_small.tile([128, 512], f32, tag="psmall")
                nc.tensor.matmul(z1_ps[:],
                                 agg0t_sb[:, :, m * 128:(m + 1) * 128],
                                 bd1_sb[:, :, cs],
                                 start=True, stop=not with_b1,
                                 perf_mode=DR)
                if with_b1:
                    nc.tensor.matmul(z1_ps[:], ones_sb[:, 0:128],
                                     b1_sb[:, cs], start=False, stop=True)
                nc.scalar.activation(act1_sb[:, m, cs], z1_ps[:], PRELU,
                                     alpha=NEG)

            dmaq = [0]

            l2_idx = [0]

            def l2_engine():
                e = L2_PATTERN[l2_idx[0] % len(L2_PATTERN)]
                l2_idx[0] += 1
                return e

            def emit_agg(c):
                a2_ps = ps_a2.tile([128, N], f32, tag="a2ps")
                a2_sb = sb_a2.tile([128, N], f32r, tag="a2sb")
                # half-major order: half 0's PSUM->SBUF copy overlaps the
                # PE's half-1 matmuls, shortening the agg->z2 chain.  The
                # copy's accum is a free sum_dst a2 for the LReLU-pool
                # linear term.
                for n in range(2):
                    cs = slice(n * 512, (n + 1) * 512)
                    for kp in range(NT // 2):
                        nc.tensor.matmul(
                            a2_ps[:, cs],
                            act1_sb[:, 2 * kp:2 * kp + 2,
                                    c * 128:(c + 1) * 128],
                            at2_sb[:, 2 * kp:2 * kp + 2, cs],
                            start=(kp == 0), stop=(kp == NT // 2 - 1),
                            perf_mode=DR,
                        )
                    acc = s_sb[:, n, c:c + 1]
                    if l2_engine() == "act":
                        nc.scalar.activation(
                            a2_sb[:, cs], a2_ps[:, cs],
                            mybir.ActivationFunctionType.Copy,
                            accum_out=acc)
                    else:
                        nc.vector.tensor_scalar(
                            a2_sb[:, cs], a2_ps[:, cs], 0.0, 0.0,
                            ALU.add, ALU.add, accum_out=acc)
                return a2_sb

            h2_queue = []   # (c, n, z2_ps) with h2 emission lagged one unit

            def emit_z2(c, n, a2_sb):
                z2_ps = ps_small.tile([128, 512], f32, tag="psmall")
                nc.tensor.matmul(
                    z2_ps[:], bd2_sb[:],
                    a2_sb[:, n * 512:(n + 1) * 512],
                    start=True, stop=not with_b2)
                if with_b2:
                    nc.tensor.matmul(z2_ps[:], b2_sb[:],
                                     ones_sb[:, 0:512], start=False,
                                     stop=True)
                h2_queue.append((c, n, z2_ps))
                if len(h2_queue) > 1:
                    emit_h2(*h2_queue.pop(0))

            def emit_h2(c, n, z2_ps):
                # single-pass sum max(z,0); the NEG * sum z linear term is
                # recovered from the a2 copy accums afterwards
                h2h = sb_h2.tile([128, 512], f8, tag="h2h")
                acc = gp_sb[:, n, c:c + 1]
                if l2_engine() == "act":
                    nc.scalar.activation(h2h[:], z2_ps[:],
                                         mybir.ActivationFunctionType.Relu,
                                         accum_out=acc)
                else:
                    nc.vector.tensor_scalar(
                        h2h[:], z2_ps[:], 0.0, 0.0, ALU.max, ALU.add,
                        accum_out=acc)

            # Per group: PE units for group G's z1 merge with the layer-2
            # PE units of group G-1; z2 lags its agg by two agg units so
            # the PE never waits on an in-flight PSUM->SBUF DMA (which
            # carries the ~900ns DMA-semaphore latency).
            pending = []   # (c, n, a2_sb) awaiting z2 emission

            def b_units(grp):
                c0 = grp * 4
                return [("agg", c0), ("agg", c0 + 1), ("agg", c0 + 2),
                        ("z2", None), ("z2", None), ("agg", c0 + 3),
                        ("z2", None), ("z2", None)]

            def run_b(unit):
                kind, c = unit
                if kind == "agg":
                    a2_sb = emit_agg(c)
                    pending.append((c, 0, a2_sb))
                    pending.append((c, 1, a2_sb))
                elif pending:
                    cc, n, a2_sb = pending.pop(0)
                    emit_z2(cc, n, a2_sb)

            for grp in range(4):
                bu = b_units(grp - 1) if grp >= 1 else []
                bi = 0
                for m in range(NT):
                    emit_z1(m, grp)
                    take = (len(bu) - bi + (NT - 1 - m)) // (NT - m)
                    for _ in range(take):
                        run_b(bu[bi])
                        bi += 1
            for unit in b_units(3):
                run_b(unit)
            while pending:
                cc, n, a2_sb = pending.pop(0)
                emit_z2(cc, n, a2_sb)
            while h2_queue:
                emit_h2(*h2_queue.pop(0))

        # ---- pooled-LReLU recombine + projection ----
        # sum_dst z2 = BD2.T @ (sum_dst a2): one 32-column matmul over the
        # copy accums, then g = (1-NEG)*sum max(z2,0) + NEG*sum z2, and
        # the two 512-halves collapse.
        with tc.tile_pool(name="ps_zs", bufs=1, space="PSUM") as ps_zs:
            s_r = work.tile([128, 2, NCHUNK], f32r)
            nc.vector.tensor_copy(s_r[:], s_sb[:])
            zs_ps = ps_zs.tile([128, 2, NCHUNK], f32)
            nc.tensor.matmul(zs_ps[:], bd2_sb[:], s_r[:],
                             start=True, stop=True)
            nc.vector.tensor_copy(zs_sb[:], zs_ps[:])
        nc.vector.tensor_scalar(gf_sb[:], zs_sb[:], NEG, 0.0,
                                ALU.mult, ALU.add)
        nc.vector.scalar_tensor_tensor(zs_sb[:], gp_sb[:], 1.0 - NEG,
                                       gf_sb[:], ALU.mult, ALU.add)
        nc.vector.tensor_tensor(gr_sb[:], zs_sb[:, 0, :], zs_sb[:, 1, :],
                                ALU.add)
        with tc.tile_pool(name="ps_y", bufs=2, space="PSUM") as ps_y:
            for half, out_sb in ((0, ye_sb), (1, yo_sb)):
                y_ps = ps_y.tile([BS // 2, D_OUT], f32, tag="yps")
                nc.tensor.matmul(y_ps[:], gr_sb[half * D2:(half + 1) * D2, :],
                                 wp_sb[half * D2:(half + 1) * D2, :],
                                 start=True, stop=not with_bp)
                if with_bp:
                    nc.tensor.matmul(
                        y_ps[:], ones_sb[:, 0:BS // 2],
                        bp_sb[:], start=False, stop=True)
                nc.scalar.activation(out_sb[:], y_ps[:], PRELU, alpha=NEG)

        y_r = y_t.rearrange("(c two) d -> two c d", two=2)
        nc.sync.dma_start(out=y_r[0, :, :], in_=ye_sb[:])
        nc.sync.dma_start(out=y_r[1, :, :], in_=yo_sb[:])


def kernel(x, edge_index, W1, b1, W2, b2, Wp, bp, _trace=False):
    x = np.ascontiguousarray(np.asarray(x, dtype=np.float32))
    consts = _build_host_constants(edge_index, W1, b1, W2, b2, Wp, bp)
    with_b1 = bool(np.any(consts["b1row"]))
    with_b2 = bool(np.any(consts["b2col"]))
    with_bp = bool(np.any(consts["bprow"]))

    nc = _build_program(with_b1, with_b2, with_bp)

    base = {"a01": consts["a01"], "a2d8": consts["a2d8"],
            "dinv2_bc": consts["dinv2_bc"], "bd1hl": consts["bd1hl"],
            "bd2": consts["bd2"], "wp_s": consts["wp_s"]}
    if with_b1:
        base["b1row"] = consts["b1row"]
    if with_b2:
        base["b2col"] = consts["b2col"]
    if with_bp:
        base["bprow"] = consts["bprow"]

    dinv = consts["dinv"]
    in_maps = [dict(base, x8=_prep_x8(x[c * BS:(c + 1) * BS], dinv))
               for c in range(NCORES)]

    from concourse.bass_utils import run_bass_kernel_spmd
    res = run_bass_kernel_spmd(nc, in_maps, core_ids=list(range(NCORES)),
                               trace=_trace)
    y = np.concatenate([res.results[c]["y"] for c in range(NCORES)], axis=0)
    out = np.ascontiguousarray(y.astype(np.float32))
    if _trace:
        return out, res
    return out
